# revision 19
# baseline (speedup 1.0000x reference)
"""Trainium2 Bass kernel for nn_MultiHeadAttention_81664508166458.

Reference computes a "cross-head" MHA: per (batch, position) the attention
matrix is HxH (H=16 heads), contracting head_dim D=128. Every position is
independent, so we shard the 8192 (batch, position) pairs across 8 cores
(1024 each), fully data-parallel, no collectives.

Host-side preprocessing (part of sharding, not timed device work):
  - weights transposed to [e_in, e_out] (k-major) and cast to bf16
  - RoPE pair permutation baked into Wq/Wk rows: head-local dim d' with
    x0 (even d) in d'=[0,64) and x1 (odd d) in d'=[64,128) so the rotation
    becomes same-partition table multiplies plus a half-swap
  - 1/sqrt(D) attention scale baked into Wq/bq
  - x transposed to [e_in, n] bf16
  - cos/sin tables and the block-diagonal softmax mask precomputed

Device pipeline per core (all matmuls bf16 with fp32 PSUM accumulation):
  1. qT/kT/vT [d, h, n] = W*T.T @ xT   (16 e-tiles x 16 k-tiles, N=512)
  2. RoPE on q,k during PSUM eviction (DVE table mults + ACT half-swap)
  3. per 8-position quad: PE computes the 128x128 "all pairs" (g,h)x(g',t)
     dot products; block-diag mask + exp (+row-sum accum) + normalize;
     PE-transpose att and the v-slice; second matmul gives O^T[d,(g,h)];
     DVE scatters into the layout-scrambled rhs for the final projection
  4. outT[r, (h,t)] = WoT.T @ scr (+bo), DMA to DRAM [E, n] (host transposes)
"""

import numpy as np
import ml_dtypes

B, S, E = 4, 2048, 2048
H, D = 16, 128
NCORES = 8
CORES_PER_BATCH = NCORES // B          # 2
NPOS = S // CORES_PER_BATCH            # 1024 positions per core
THETA = 10000.0
MASK_NEG = -30000.0

BF16 = ml_dtypes.bfloat16

# ---------------------------------------------------------------------------
# Host-side preprocessing
# ---------------------------------------------------------------------------


def _rope_perm():
    """P_IDX[new] = old row index: x0 (even d) -> d'=[0,64), x1 (odd) -> [64,128)."""
    p = np.empty(E, np.int64)
    for h in range(H):
        base = h * D
        i = np.arange(D // 2)
        p[base + i] = base + 2 * i
        p[base + 64 + i] = base + 2 * i + 1
    return p


def _rope_tables(npos, offset):
    """cos table C[p, n] and signed sin table S[p, n], p in [0,128)."""
    inv = 1.0 / (THETA ** (np.arange(0, D, 2, dtype=np.float64) / D))  # [64]
    pos = np.arange(offset, offset + npos, dtype=np.float64)
    fr = np.outer(inv, pos)  # [64, npos]
    c = np.cos(fr).astype(np.float32)
    s = np.sin(fr).astype(np.float32)
    cos_b = np.concatenate([c, c], axis=0)            # [128, npos]
    sin_b = np.concatenate([-s, s], axis=0)           # signed
    return np.ascontiguousarray(cos_b), np.ascontiguousarray(sin_b)


def _blockdiag_mask():
    m = np.full((128, 128), MASK_NEG, np.float32)
    for g in range(8):
        m[g * 16:(g + 1) * 16, g * 16:(g + 1) * 16] = 0.0
    return m


def prepare_host(x, Wq, bq, Wk, bk, Wv, bv, Wo, bo, npos=NPOS, ncores=NCORES):
    """Returns (shared weight arrays dict, list of per-core in_maps)."""
    x = np.asarray(x, np.float32)
    perm = _rope_perm()
    scale = np.float32(1.0 / np.sqrt(D))

    wqt = np.ascontiguousarray((np.asarray(Wq, np.float32)[perm, :] * scale).T).astype(BF16)
    wkt = np.ascontiguousarray(np.asarray(Wk, np.float32)[perm, :].T).astype(BF16)
    wvt = np.ascontiguousarray(np.asarray(Wv, np.float32).T).astype(BF16)
    wot = np.ascontiguousarray(np.asarray(Wo, np.float32).T).astype(BF16)
    bq_p = (np.asarray(bq, np.float32)[perm] * scale).copy()
    bk_p = np.asarray(bk, np.float32)[perm].copy()
    bv_p = np.asarray(bv, np.float32).copy()
    bo_p = np.asarray(bo, np.float32).copy()
    mask = _blockdiag_mask()

    in_maps = []
    meta = []
    for c in range(ncores):
        bc = c // CORES_PER_BATCH
        o = (c % CORES_PER_BATCH) * npos
        xc = x[bc, o:o + npos, :]                      # [npos, E]
        xt = np.ascontiguousarray(xc.T).astype(BF16)   # [E, npos]
        cos_b, sin_b = _rope_tables(npos, o)
        in_maps.append({
            "xt": xt, "wqt": wqt, "wkt": wkt, "wvt": wvt, "wot": wot,
            "bq": bq_p, "bk": bk_p, "bv": bv_p, "bo": bo_p,
            "cosb": cos_b, "sinb": sin_b, "mask": mask,
        })
        meta.append((bc, o))
    return in_maps, meta


def assemble_output(outs, meta, npos=NPOS):
    """outs: list of per-core {'outt': [E, npos] f32}. Returns [B, S, E]."""
    full = np.empty((B, S, E), np.float32)
    tw = npos // 16
    for (bc, o), res in zip(meta, outs):
        outt = res["outt"]                       # [E, h*tw+tc]
        # row s' = h*128 + (o//16 + tc), value = outt[:, h*tw+tc]
        v = outt.reshape(E, H, tw)               # [E, h, tc]
        v = np.transpose(v, (1, 2, 0))           # [h, tc, E]
        t0 = o // 16
        for h in range(H):
            full[bc, h * 128 + t0: h * 128 + t0 + tw, :] = v[h]
    return full


# ---------------------------------------------------------------------------
# Numpy emulator of the exact device dataflow (index-math validation)
# ---------------------------------------------------------------------------


def emulate_core(im, npos=NPOS):
    f32 = np.float32
    xt = im["xt"].astype(f32)
    qT = (im["wqt"].astype(f32).T @ xt) + im["bq"][:, None]   # [E, n]
    kT = (im["wkt"].astype(f32).T @ xt) + im["bk"][:, None]
    vT = (im["wvt"].astype(f32).T @ xt) + im["bv"][:, None]
    C, Sg = im["cosb"].astype(f32), im["sinb"].astype(f32)

    def rope(t):
        t3 = t.reshape(H, D, npos)                            # [h, d', n]
        sw = np.concatenate([t3[:, 64:, :], t3[:, :64, :]], axis=1)
        r = t3 * C[None] + sw * Sg[None]
        return r.astype(BF16).astype(f32)

    qr, kr = rope(qT), rope(kT)
    vb = vT.astype(BF16).astype(f32).reshape(H, D, npos)
    scr = np.zeros((D, 16, npos), f32)                        # [d, j, h*tw+tc]
    tw = npos // 16
    for g0 in range(npos // 8):
        n0 = 8 * g0
        j0, tc = n0 % 16, g0 // 2
        q_blk = qr[:, :, n0:n0 + 8]                           # [h, d, g]
        k_blk = kr[:, :, n0:n0 + 8]
        lhs = np.transpose(q_blk, (1, 2, 0)).reshape(D, 128)  # [d, (g,h)]
        rhs = np.transpose(k_blk, (1, 2, 0)).reshape(D, 128)  # [d, (g,t)]
        qk = lhs.T @ rhs + im["mask"]
        e = np.exp(qk)
        att = (e / e.sum(1, keepdims=True)).astype(BF16).astype(f32)
        vm = np.transpose(vb[:, :, n0:n0 + 8], (1, 2, 0)).reshape(D, 128)  # [d,(g,t)]
        # out2T[d, (g,h)] = sum_{(g,t)} vm[d, (g,t)] * att[(g,h), (g,t)]
        o2 = vm @ att.T                 # [d, (g,h)]
        o2v = o2.reshape(D, 8, 16)
        scr.reshape(D, 16, H, tw)[:, j0:j0 + 8, :, tc] = o2v
    # scr[d, j, col] -> rhs row e=(j*128+d)
    rhs_full = np.transpose(scr, (1, 0, 2)).reshape(16 * D, npos).astype(BF16).astype(f32)
    outt = im["wot"].astype(f32).T @ rhs_full + im["bo"][:, None]
    return {"outt": outt.astype(f32)}


def emulate_full(inputs, npos=NPOS, ncores=NCORES):
    in_maps, meta = prepare_host(**inputs, npos=npos, ncores=ncores)
    outs = [emulate_core(im, npos) for im in in_maps]
    return assemble_output(outs, meta, npos)


# ---------------------------------------------------------------------------
# Bass kernel
# ---------------------------------------------------------------------------

_NC_CACHE = {}


def build_nc(npos=NPOS, reps=1):
    import concourse.bass as bass
    import concourse.tile as tile
    from concourse import bacc, mybir
    from concourse.masks import make_identity

    key = (npos, reps)
    if key in _NC_CACHE:
        return _NC_CACHE[key]

    f32, bf16 = mybir.dt.float32, mybir.dt.bfloat16
    CH = min(512, npos)          # free-dim chunk (one PSUM bank fp32)
    NCH = npos // CH
    TW = npos // 16

    nc = bacc.Bacc("TRN2", target_bir_lowering=False, debug=False)

    xt_d = nc.dram_tensor("xt", [E, npos], bf16, kind="ExternalInput")
    w_d = {
        "q": nc.dram_tensor("wqt", [E, E], bf16, kind="ExternalInput"),
        "k": nc.dram_tensor("wkt", [E, E], bf16, kind="ExternalInput"),
        "v": nc.dram_tensor("wvt", [E, E], bf16, kind="ExternalInput"),
        "o": nc.dram_tensor("wot", [E, E], bf16, kind="ExternalInput"),
    }
    b_d = {
        "q": nc.dram_tensor("bq", [E], f32, kind="ExternalInput"),
        "k": nc.dram_tensor("bk", [E], f32, kind="ExternalInput"),
        "v": nc.dram_tensor("bv", [E], f32, kind="ExternalInput"),
        "o": nc.dram_tensor("bo", [E], f32, kind="ExternalInput"),
    }
    cos_d = nc.dram_tensor("cosb", [128, npos], f32, kind="ExternalInput")
    sin_d = nc.dram_tensor("sinb", [128, npos], f32, kind="ExternalInput")
    mask_d = nc.dram_tensor("mask", [128, 128], f32, kind="ExternalInput")
    out_d = nc.dram_tensor("outt", [E, npos], f32, kind="ExternalOutput")

    Exp = mybir.ActivationFunctionType.Exp
    Ident = mybir.ActivationFunctionType.Identity

    def body(tc):
        with (
            tc.tile_pool(name="consts", bufs=1) as consts,
            tc.tile_pool(name="wpool", bufs=2) as wpool,
            tc.tile_pool(name="scrp", bufs=1) as scrp,
            tc.tile_pool(name="tmp", bufs=2) as tmp,
            tc.tile_pool(name="attp", bufs=3) as attp,
            tc.tile_pool(name="outp", bufs=2) as outp,
        ):
            cos_sb = consts.tile([128, npos], f32)
            nc.sync.dma_start(cos_sb, cos_d.ap())
            sin_sb = consts.tile([128, npos], f32)
            nc.sync.dma_start(sin_sb, sin_d.ap())
            mask_sb = consts.tile([128, 128], f32)
            nc.sync.dma_start(mask_sb, mask_d.ap())
            ident = consts.tile([128, 128], bf16)
            make_identity(nc, ident)
            b_sb = {}
            for p in ("q", "k", "v", "o"):
                b_sb[p] = consts.tile([128, 16], f32, tag=f"b_{p}", name=f"b_{p}")
                nc.sync.dma_start(b_sb[p], b_d[p].ap().rearrange("(t p) -> p t", p=128))

            scr_sb = scrp.tile([128, 16, npos], bf16)

            with tc.tile_pool(name="qkvp", bufs=1) as qkvp:
                # layout [d, n, h]: per-quad (g,h)/(g,t) views are contiguous
                qkv_sb = {
                    p: qkvp.tile([128, npos, 16], bf16, tag=f"qkv_{p}", name=f"qkv_{p}")
                    for p in ("q", "k", "v")
                }

                # ---------------- phase 1: projections ----------------
                with (
                    tc.tile_pool(name="xp", bufs=1) as xp,
                    tc.tile_pool(name="ps1", bufs=3, space="PSUM") as ps1,
                ):
                    xt_sb = xp.tile([128, 16, npos], bf16)
                    nc.sync.dma_start(
                        xt_sb, xt_d.ap().rearrange("(kt kp) n -> kp kt n", kp=128))

                    for p in ("q", "k", "v"):
                        wv_d = w_d[p].ap().rearrange("(kt kp) e -> kp kt e", kp=128)
                        for mg in range(8):
                            w_sb = wpool.tile([128, 16, 256], bf16, tag="w")
                            nc.sync.dma_start(
                                w_sb, wv_d[:, :, mg * 256:(mg + 1) * 256])
                            for mo in range(2):
                                m = mg * 2 + mo
                                for ch in range(NCH):
                                    csl = slice(ch * CH, (ch + 1) * CH)
                                    ps = ps1.tile([128, CH], f32)
                                    for kt in range(16):
                                        nc.tensor.matmul(
                                            ps,
                                            lhsT=w_sb[:, kt, mo * 128:(mo + 1) * 128],
                                            rhs=xt_sb[:, kt, csl],
                                            start=(kt == 0), stop=(kt == 15))
                                    if p == "v":
                                        nc.scalar.activation(
                                            qkv_sb["v"][:, csl, m], ps, Ident,
                                            bias=b_sb["v"][:, m:m + 1])
                                    else:
                                        nc.vector.tensor_scalar_add(
                                            ps, ps, b_sb[p][:, m:m + 1])
                                        t1 = tmp.tile([128, CH], f32, tag="t1")
                                        nc.vector.tensor_mul(t1, ps, cos_sb[:, csl])
                                        tsw = tmp.tile([128, CH], f32, tag="tsw")
                                        nc.scalar.copy(tsw[0:64, :], ps[64:128, :])
                                        nc.scalar.copy(tsw[64:128, :], ps[0:64, :])
                                        nc.vector.tensor_mul(tsw, tsw, sin_sb[:, csl])
                                        nc.vector.tensor_add(
                                            qkv_sb[p][:, csl, m], t1, tsw)

                # ---------------- phase 2: attention ----------------
                scr4 = scr_sb.rearrange("p j (h t) -> p j h t", h=16)
                with tc.tile_pool(name="ps2", bufs=2, space="PSUM") as ps2:
                    for g0 in range(npos // 8):
                        n0 = 8 * g0
                        j0, tc_ = n0 % 16, g0 // 2
                        q_v = qkv_sb["q"][:, n0:n0 + 8, :].rearrange("d g h -> d (g h)")
                        k_v = qkv_sb["k"][:, n0:n0 + 8, :].rearrange("d g h -> d (g h)")
                        qk_ps = ps2.tile([128, 128], f32, tag="qk")
                        nc.tensor.matmul(qk_ps, lhsT=q_v, rhs=k_v, start=True, stop=True)
                        nc.vector.tensor_add(qk_ps, qk_ps, mask_sb)
                        att = attp.tile([128, 128], bf16, tag="att")
                        rs = attp.tile([128, 1], f32, tag="rs")
                        nc.scalar.activation(att, qk_ps, Exp, accum_out=rs)
                        rc = attp.tile([128, 1], f32, tag="rc")
                        nc.vector.reciprocal(rc, rs)
                        nc.vector.tensor_scalar_mul(att, att, rc)

                        attT_ps = ps2.tile([128, 128], bf16, tag="attT")
                        nc.tensor.transpose(attT_ps, att, ident)
                        attT = attp.tile([128, 128], bf16, tag="attTs")
                        nc.vector.tensor_copy(attT, attT_ps)

                        v_v = qkv_sb["v"][:, n0:n0 + 8, :].rearrange("d g t -> d (g t)")
                        vp_ps = ps2.tile([128, 128], bf16, tag="vp")
                        nc.tensor.transpose(vp_ps, v_v, ident)
                        vp = attp.tile([128, 128], bf16, tag="vps")
                        nc.vector.tensor_copy(vp, vp_ps)

                        o_ps = ps2.tile([128, 128], f32, tag="o")
                        nc.tensor.matmul(o_ps, lhsT=vp, rhs=attT, start=True, stop=True)
                        nc.vector.tensor_copy(
                            scr4[:, j0:j0 + 8, :, tc_:tc_ + 1],
                            o_ps.rearrange("p (g h) -> p g h", g=8))

            # ---------------- phase 3: output projection ----------------
            wo_v = w_d["o"].ap().rearrange("(jt jp) r -> jp jt r", jp=128)
            out_v = out_d.ap().rearrange("(rt rp) n -> rp rt n", rp=128)
            with tc.tile_pool(name="ps3", bufs=3, space="PSUM") as ps3:
                for rg in range(8):
                    w_sb = wpool.tile([128, 16, 256], bf16, tag="w")
                    nc.sync.dma_start(w_sb, wo_v[:, :, rg * 256:(rg + 1) * 256])
                    for ro in range(2):
                        r = rg * 2 + ro
                        for ch in range(NCH):
                            csl = slice(ch * CH, (ch + 1) * CH)
                            ps = ps3.tile([128, CH], f32)
                            for j in range(16):
                                nc.tensor.matmul(
                                    ps,
                                    lhsT=w_sb[:, j, ro * 128:(ro + 1) * 128],
                                    rhs=scr_sb[:, j, csl],
                                    start=(j == 0), stop=(j == 15))
                            o_sb = outp.tile([128, CH], f32)
                            nc.scalar.activation(
                                o_sb, ps, Ident, bias=b_sb["o"][:, r:r + 1])
                            nc.sync.dma_start(out_v[:, r, csl], o_sb)

    with tile.TileContext(nc) as tc:
        for _ in range(reps):
            body(tc)

    nc.compile()
    _NC_CACHE[key] = nc
    return nc


# ---------------------------------------------------------------------------
# Runner (PJRT via axon, cached jitted callable)
# ---------------------------------------------------------------------------

_RUNNER_CACHE = {}


def make_runner(nc, ncores=NCORES):
    """Returns run(in_maps) -> list of per-core output dicts.

    Mirrors bass2jax.run_bass_via_pjrt but caches the jitted callable and
    does NOT donate output buffers (kernel writes every output element), so
    repeated timed calls don't re-trace or re-transfer.
    """
    key = id(nc)
    if key in _RUNNER_CACHE:
        return _RUNNER_CACHE[key]

    import jax
    import numpy as _np
    from jax.sharding import Mesh, PartitionSpec
    from jax.experimental.shard_map import shard_map
    from concourse import mybir
    from concourse import bass2jax
    from concourse.bass2jax import (
        _bass_exec_p, install_neuronx_cc_hook, partition_id_tensor)

    install_neuronx_cc_hook()

    partition_name = (
        nc.partition_id_tensor.name if nc.partition_id_tensor else None)
    in_names, out_names, out_avals, zero_outs = [], [], [], []
    for alloc in nc.m.functions[0].allocations:
        if not isinstance(alloc, mybir.MemoryLocationSet):
            continue
        name = alloc.memorylocations[0].name
        if alloc.kind == "ExternalInput":
            if name == partition_name:
                continue
            in_names.append(name)
        elif alloc.kind == "ExternalOutput":
            shape = tuple(alloc.tensor_shape)
            dtype = mybir.dt.np(alloc.dtype)
            out_names.append(name)
            out_avals.append(jax.core.ShapedArray(shape, dtype))
            zero_outs.append(_np.zeros(shape, dtype))
    n_params = len(in_names)
    all_in_names = in_names + out_names
    if partition_name is not None:
        all_in_names = all_in_names + [partition_name]

    def _body(*args):
        operands = list(args)
        if partition_name is not None:
            operands.append(partition_id_tensor())
        outs = _bass_exec_p.bind(
            *operands,
            out_avals=tuple(out_avals),
            in_names=tuple(all_in_names),
            out_names=tuple(out_names),
            lowering_input_output_aliases=(),
            sim_require_finite=True,
            sim_require_nnan=True,
            nc=nc,
        )
        return tuple(outs)

    devices = jax.devices()[:ncores]
    mesh = Mesh(np.asarray(devices), ("core",))
    n_outs = len(out_names)
    jitted = jax.jit(
        shard_map(
            _body, mesh=mesh,
            in_specs=(PartitionSpec("core"),) * (n_params + n_outs),
            out_specs=(PartitionSpec("core"),) * n_outs,
            check_rep=False,
        ),
        keep_unused=True,
    )

    zeros_dev = [
        jax.device_put(
            _np.zeros((ncores * z.shape[0], *z.shape[1:]), z.dtype))
        for z in zero_outs
    ]

    def put(in_maps):
        concat = [
            _np.concatenate([_np.asarray(m[name]) for m in in_maps], axis=0)
            for name in in_names
        ]
        return [jax.device_put(a) for a in concat]

    def run_dev(in_dev):
        outs = jitted(*in_dev, *zeros_dev)
        jax.block_until_ready(outs)
        return outs

    def run(in_maps):
        outs = run_dev(put(in_maps))
        res = []
        for c in range(len(in_maps)):
            res.append({
                name: _np.asarray(outs[i]).reshape(
                    len(in_maps), *out_avals[i].shape)[c]
                for i, name in enumerate(out_names)
            })
        return res

    run.put = put
    run.run_dev = run_dev
    run.out_names = out_names
    _RUNNER_CACHE[key] = run
    return run


def kernel(**inputs) -> np.ndarray:
    in_maps, meta = prepare_host(**{k: np.asarray(v) for k, v in inputs.items()})
    nc = build_nc(NPOS)
    run = make_runner(nc, NCORES)
    outs = run(in_maps)
    return assemble_output(outs, meta, NPOS)


# revision 49
# speedup vs baseline: 7.8266x; 7.8266x over previous
"""Trainium2 Bass kernel for nn_MultiHeadAttention_81664508166458.

Reference computes a "cross-head" MHA: per (batch, position) the attention
matrix is HxH (H=16 heads), contracting head_dim D=128. Every position is
independent, so we shard the 8192 (batch, position) pairs across 8 cores
(1024 each), fully data-parallel, no collectives.

Host-side preprocessing (part of sharding, not timed device work):
  - weights transposed to [e_in, e_out] (k-major) and cast to bf16
  - RoPE pair permutation baked into Wq/Wk rows: head-local dim d' with
    x0 (even d) in d'=[0,64) and x1 (odd d) in d'=[64,128) so the rotation
    becomes same-partition table multiplies plus a half-swap
  - 1/sqrt(D) attention scale baked into Wq/bq
  - x transposed to [e_in, n] bf16
  - cos/sin tables and the block-diagonal softmax mask precomputed

Device pipeline per core (all matmuls bf16 with fp32 PSUM accumulation):
  1. qT/kT/vT [d, h, n] = W*T.T @ xT   (16 e-tiles x 16 k-tiles, N=512)
  2. RoPE on q,k during PSUM eviction (DVE table mults + ACT half-swap)
  3. per 8-position quad: PE computes the 128x128 "all pairs" (g,h)x(g',t)
     dot products; block-diag mask + exp (+row-sum accum) + normalize;
     PE-transpose att and the v-slice; second matmul gives O^T[d,(g,h)];
     DVE scatters into the layout-scrambled rhs for the final projection
  4. outT[r, (h,t)] = WoT.T @ scr (+bo), DMA to DRAM [E, n] (host transposes)
"""

import numpy as np
import ml_dtypes

B, S, E = 4, 2048, 2048
H, D = 16, 128
NCORES = 8
CORES_PER_BATCH = NCORES // B          # 2
NPOS = S // CORES_PER_BATCH            # 1024 positions per core
THETA = 10000.0
MASK_NEG = -30000.0

BF16 = ml_dtypes.bfloat16

# ---------------------------------------------------------------------------
# Host-side preprocessing
# ---------------------------------------------------------------------------


def _rope_perm():
    """P_IDX[new] = old row index: x0 (even d) -> d'=[0,64), x1 (odd) -> [64,128)."""
    p = np.empty(E, np.int64)
    for h in range(H):
        base = h * D
        i = np.arange(D // 2)
        p[base + i] = base + 2 * i
        p[base + 64 + i] = base + 2 * i + 1
    return p


def _rope_tables(npos, offset):
    """cos table C[p, n] and signed sin table S[p, n], p in [0,128)."""
    inv = 1.0 / (THETA ** (np.arange(0, D, 2, dtype=np.float64) / D))  # [64]
    pos = np.arange(offset, offset + npos, dtype=np.float64)
    fr = np.outer(inv, pos)  # [64, npos]
    c = np.cos(fr).astype(np.float32)
    s = np.sin(fr).astype(np.float32)
    cos_b = np.concatenate([c, c], axis=0)            # [128, npos]
    sin_b = np.concatenate([-s, s], axis=0)           # signed
    return np.ascontiguousarray(cos_b), np.ascontiguousarray(sin_b)


def _blockdiag_mask():
    m = np.full((128, 128), MASK_NEG, np.float32)
    for g in range(8):
        m[g * 16:(g + 1) * 16, g * 16:(g + 1) * 16] = 0.0
    return m


# exact bf16-representable mask magnitude (softmax is shift-invariant, but we
# keep the on-diagonal shift exactly zero: +MASKVAL via matmul, -MASKVAL bias)
MASKVAL = float(np.float32(BF16(30000.0)))


def _mask_mm():
    """K=8 rank-8 matmul operands adding +MASKVAL on the block diagonal.
    maskl[g, p] = MASKVAL if p//16==g else 0 ; maskr[g, f] = 1 if f//16==g."""
    ind = np.zeros((8, 128), np.float32)
    for g in range(8):
        ind[g, g * 16:(g + 1) * 16] = 1.0
    return (ind * MASKVAL).astype(BF16), ind.astype(BF16)


def prepare_host(x, Wq, bq, Wk, bk, Wv, bv, Wo, bo, npos=NPOS, ncores=NCORES):
    """Returns (shared weight arrays dict, list of per-core in_maps)."""
    x = np.asarray(x, np.float32)
    perm = _rope_perm()
    scale = np.float32(1.0 / np.sqrt(D))

    wqt = np.ascontiguousarray((np.asarray(Wq, np.float32)[perm, :] * scale).T).astype(BF16)
    wkt = np.ascontiguousarray(np.asarray(Wk, np.float32)[perm, :].T).astype(BF16)
    wvt = np.ascontiguousarray(np.asarray(Wv, np.float32).T).astype(BF16)
    wot = np.ascontiguousarray(np.asarray(Wo, np.float32).T).astype(BF16)
    bq_p = (np.asarray(bq, np.float32)[perm] * scale).copy()
    bk_p = np.asarray(bk, np.float32)[perm].copy()
    bv_p = np.asarray(bv, np.float32).copy()
    bo_p = np.asarray(bo, np.float32).copy()
    mask = _blockdiag_mask()
    maskl, maskr = _mask_mm()

    in_maps = []
    meta = []
    for c in range(ncores):
        bc = c // CORES_PER_BATCH
        o = (c % CORES_PER_BATCH) * npos
        xc = x[bc, o:o + npos, :]                      # [npos, E]
        xt = np.ascontiguousarray(xc.T).astype(BF16)   # [E, npos]
        cos_b, sin_b = _rope_tables(npos, o)
        in_maps.append({
            "xt": xt, "wqt": wqt, "wkt": wkt, "wvt": wvt, "wot": wot,
            "bq": bq_p, "bk": bk_p, "bv": bv_p, "bo": bo_p,
            "cosb": cos_b, "sinb": sin_b, "mask": mask,
            "maskl": maskl, "maskr": maskr,
        })
        meta.append((bc, o))
    return in_maps, meta


def assemble_output(outs, meta, npos=NPOS, layout="h_t"):
    """outs: list of per-core {'outt': [E, npos] f32}. Returns [B, S, E].

    layout "h_t": outt col = h*tw + tc (tc local).
    layout "t_h": outt col = tc*16 + h (scrsplit build).
    """
    full = np.empty((B, S, E), np.float32)
    tw = npos // 16
    for (bc, o), res in zip(meta, outs):
        outt = res["outt"]
        if layout == "h_t":
            v = outt.reshape(E, H, tw)           # [E, h, tc]
            v = np.transpose(v, (1, 2, 0))       # [h, tc, E]
        else:
            v = outt.reshape(E, tw, H)           # [E, tc, h]
            v = np.transpose(v, (2, 1, 0))       # [h, tc, E]
        t0 = o // 16
        for h in range(H):
            full[bc, h * 128 + t0: h * 128 + t0 + tw, :] = v[h]
    return full


# ---------------------------------------------------------------------------
# Numpy emulator of the exact device dataflow (index-math validation)
# ---------------------------------------------------------------------------


def emulate_core(im, npos=NPOS, layout="h_t"):
    f32 = np.float32
    xt = im["xt"].astype(f32)
    qT = (im["wqt"].astype(f32).T @ xt) + im["bq"][:, None]   # [E, n]
    kT = (im["wkt"].astype(f32).T @ xt) + im["bk"][:, None]
    vT = (im["wvt"].astype(f32).T @ xt) + im["bv"][:, None]
    C, Sg = im["cosb"].astype(f32), im["sinb"].astype(f32)

    def rope(t):
        t3 = t.reshape(H, D, npos)                            # [h, d', n]
        sw = np.concatenate([t3[:, 64:, :], t3[:, :64, :]], axis=1)
        r = t3 * C[None] + sw * Sg[None]
        return r.astype(BF16).astype(f32)

    qr, kr = rope(qT), rope(kT)
    vb = vT.astype(BF16).astype(f32).reshape(H, D, npos)
    scr = np.zeros((D, 16, npos), f32)                        # [d, j, h*tw+tc]
    tw = npos // 16
    for g0 in range(npos // 8):
        n0 = 8 * g0
        j0, tc = n0 % 16, g0 // 2
        q_blk = qr[:, :, n0:n0 + 8]                           # [h, d, g]
        k_blk = kr[:, :, n0:n0 + 8]
        lhs = np.transpose(q_blk, (1, 2, 0)).reshape(D, 128)  # [d, (g,h)]
        rhs = np.transpose(k_blk, (1, 2, 0)).reshape(D, 128)  # [d, (g,t)]
        qk = lhs.T @ rhs + im["mask"]
        e = np.exp(qk)
        att = (e / e.sum(1, keepdims=True)).astype(BF16).astype(f32)
        vm = np.transpose(vb[:, :, n0:n0 + 8], (1, 2, 0)).reshape(D, 128)  # [d,(g,t)]
        # out2T[d, (g,h)] = sum_{(g,t)} vm[d, (g,t)] * att[(g,h), (g,t)]
        o2 = vm @ att.T                 # [d, (g,h)]
        o2v = o2.reshape(D, 8, 16)
        if layout == "h_t":
            scr.reshape(D, 16, H, tw)[:, j0:j0 + 8, :, tc] = o2v
        else:
            scr.reshape(D, 16, tw, H)[:, j0:j0 + 8, tc, :] = o2v
    # scr[d, j, col] -> rhs row e=(j*128+d)
    rhs_full = np.transpose(scr, (1, 0, 2)).reshape(16 * D, npos).astype(BF16).astype(f32)
    outt = im["wot"].astype(f32).T @ rhs_full + im["bo"][:, None]
    return {"outt": outt.astype(f32)}


def emulate_full(inputs, npos=NPOS, ncores=NCORES, layout="h_t"):
    in_maps, meta = prepare_host(**inputs, npos=npos, ncores=ncores)
    outs = [emulate_core(im, npos, layout) for im in in_maps]
    return assemble_output(outs, meta, npos, layout)


# ---------------------------------------------------------------------------
# Bass kernel
# ---------------------------------------------------------------------------

_NC_CACHE = {}


def build_nc(npos=NPOS, reps=1, opts=frozenset()):
    import concourse.bass as bass
    import concourse.tile as tile
    from concourse import bacc, mybir
    from concourse.masks import make_identity

    opts = frozenset(opts)
    key = (npos, reps, opts)
    if key in _NC_CACHE:
        return _NC_CACHE[key]

    f32, bf16 = mybir.dt.float32, mybir.dt.bfloat16
    CH = min(512, npos)          # free-dim chunk (one PSUM bank fp32)
    NCH = npos // CH
    TW = npos // 16

    nc = bacc.Bacc("TRN2", target_bir_lowering=False, debug=False)

    xt_d = nc.dram_tensor("xt", [E, npos], bf16, kind="ExternalInput")
    w_d = {
        "q": nc.dram_tensor("wqt", [E, E], bf16, kind="ExternalInput"),
        "k": nc.dram_tensor("wkt", [E, E], bf16, kind="ExternalInput"),
        "v": nc.dram_tensor("wvt", [E, E], bf16, kind="ExternalInput"),
        "o": nc.dram_tensor("wot", [E, E], bf16, kind="ExternalInput"),
    }
    b_d = {
        "q": nc.dram_tensor("bq", [E], f32, kind="ExternalInput"),
        "k": nc.dram_tensor("bk", [E], f32, kind="ExternalInput"),
        "v": nc.dram_tensor("bv", [E], f32, kind="ExternalInput"),
        "o": nc.dram_tensor("bo", [E], f32, kind="ExternalInput"),
    }
    cos_d = nc.dram_tensor("cosb", [128, npos], f32, kind="ExternalInput")
    sin_d = nc.dram_tensor("sinb", [128, npos], f32, kind="ExternalInput")
    mask_d = nc.dram_tensor("mask", [128, 128], f32, kind="ExternalInput")
    maskl_d = nc.dram_tensor("maskl", [8, 128], bf16, kind="ExternalInput")
    maskr_d = nc.dram_tensor("maskr", [8, 128], bf16, kind="ExternalInput")
    out_d = nc.dram_tensor("outt", [E, npos], f32, kind="ExternalOutput")

    Exp = mybir.ActivationFunctionType.Exp
    Ident = mybir.ActivationFunctionType.Identity

    def body(tc):
        with (
            tc.tile_pool(name="consts", bufs=1) as consts,
            tc.tile_pool(name="wpool", bufs=2) as wpool,
            tc.tile_pool(name="scrp", bufs=1) as scrp,
            tc.tile_pool(name="tmp", bufs=2) as tmp,
            tc.tile_pool(name="attp", bufs=3) as attp,
            tc.tile_pool(name="outp", bufs=2) as outp,
        ):
            cos_sb = consts.tile([128, npos], f32)
            nc.sync.dma_start(cos_sb, cos_d.ap())
            sin_sb = consts.tile([128, npos], f32)
            nc.sync.dma_start(sin_sb, sin_d.ap())
            if "fastmask" in opts:
                ml_sb = consts.tile([8, 128], bf16)
                nc.sync.dma_start(ml_sb, maskl_d.ap())
                mr_sb = consts.tile([8, 128], bf16)
                nc.sync.dma_start(mr_sb, maskr_d.ap())
                ebias = consts.tile([128, 1], f32)
                nc.vector.memset(ebias, -MASKVAL)
            else:
                mask_sb = consts.tile([128, 128], f32)
                nc.sync.dma_start(mask_sb, mask_d.ap())
            ident = consts.tile([128, 128], bf16)
            make_identity(nc, ident)
            b_sb = {}
            for p in ("q", "k", "v", "o"):
                b_sb[p] = consts.tile([128, 16], f32, tag=f"b_{p}", name=f"b_{p}")
                nc.sync.dma_start(b_sb[p], b_d[p].ap().rearrange("(t p) -> p t", p=128))

            if "scrsplit" in opts:
                scrA = scrp.tile([128, 16, npos // 2], bf16, tag="scrA")
                scrB = scrp.tile([128, 16, npos // 2], bf16, tag="scrB")
            else:
                scr_sb = scrp.tile([128, 16, npos], bf16)

            with tc.tile_pool(name="qkvp", bufs=1) as qkvp:
                # layout [d, n, h]: per-quad (g,h)/(g,t) views are contiguous
                qkv_sb = {
                    p: qkvp.tile([128, npos, 16], bf16, tag=f"qkv_{p}", name=f"qkv_{p}")
                    for p in ("q", "k", "v")
                }

                # ---------------- phase 1: projections ----------------
                with (
                    tc.tile_pool(name="xp", bufs=1) as xp,
                    tc.tile_pool(
                        name="ps1", bufs=4 if "ps1b4" in opts else 3,
                        space="PSUM") as ps1,
                ):
                    xt_sb = xp.tile([128, 16, npos], bf16)
                    nc.sync.dma_start(
                        xt_sb, xt_d.ap().rearrange("(kt kp) n -> kp kt n", kp=128))

                    for p in ("q", "k", "v"):
                        wv_d = w_d[p].ap().rearrange("(kt kp) e -> kp kt e", kp=128)
                        for mg in range(8):
                            w_sb = wpool.tile([128, 16, 256], bf16, tag="w")
                            nc.sync.dma_start(
                                w_sb, wv_d[:, :, mg * 256:(mg + 1) * 256])
                            for mo in range(2):
                                m = mg * 2 + mo
                                for ch in range(NCH):
                                    csl = slice(ch * CH, (ch + 1) * CH)
                                    ps = ps1.tile([128, CH], f32)
                                    for kt in range(16):
                                        nc.tensor.matmul(
                                            ps,
                                            lhsT=w_sb[:, kt, mo * 128:(mo + 1) * 128],
                                            rhs=xt_sb[:, kt, csl],
                                            start=(kt == 0), stop=(kt == 15))
                                    if p == "v" or "norope" in opts:
                                        nc.scalar.activation(
                                            qkv_sb[p][:, csl, m], ps, Ident,
                                            bias=b_sb["v"][:, m:m + 1])
                                    else:
                                        nc.vector.tensor_scalar_add(
                                            ps, ps, b_sb[p][:, m:m + 1])
                                        t1 = tmp.tile([128, CH], f32, tag="t1")
                                        nc.vector.tensor_mul(t1, ps, cos_sb[:, csl])
                                        tsw = tmp.tile([128, CH], f32, tag="tsw")
                                        nc.scalar.copy(tsw[0:64, :], ps[64:128, :])
                                        nc.scalar.copy(tsw[64:128, :], ps[0:64, :])
                                        nc.vector.tensor_mul(tsw, tsw, sin_sb[:, csl])
                                        nc.vector.tensor_add(
                                            qkv_sb[p][:, csl, m], t1, tsw)

                # ---------------- phase 2: attention ----------------
                if "scrsplit" in opts:
                    scr5 = [
                        s.rearrange("p j (t h) -> p j t h", h=16)
                        for s in (scrA, scrB)
                    ]
                else:
                    scr4 = scr_sb.rearrange("p j (h t) -> p j h t", h=16)
                nquads = npos // 8
                vphoist = "vphoist" in opts

                with tc.tile_pool(name="v2p", bufs=1) as v2p:
                    if vphoist:
                        v2_sb = v2p.tile([128, nquads, 128], bf16)
                        with tc.tile_pool(
                                name="vpps", bufs=4, space="PSUM") as vpps:
                            for g0 in range(nquads):
                                n0 = 8 * g0
                                v_v = qkv_sb["v"][:, n0:n0 + 8, :].rearrange(
                                    "d g t -> d (g t)")
                                vp_ps = vpps.tile([128, 128], bf16, tag="vp")
                                nc.tensor.transpose(vp_ps, v_v, ident)
                                nc.vector.tensor_copy(v2_sb[:, g0, :], vp_ps)

                    if vphoist:
                        ps2_cm = tc.tile_pool(name="ps2", bufs=3, space="PSUM")
                    else:
                        ps2_cm = tc.tile_pool(name="ps2", bufs=2, space="PSUM")
                    with ps2_cm as ps2:
                        if "noatt" in opts:
                            if "scrsplit" in opts:
                                nc.vector.memset(scrA, 0.0)
                                nc.vector.memset(scrB, 0.0)
                            else:
                                nc.vector.memset(scr_sb, 0.0)
                        if "qbatch" in opts:
                            assert {"fastmask", "scrsplit"} <= opts
                            tw2 = TW // 2
                            for a in range(nquads // 4):
                                qk4 = ps2.tile(
                                    [128, 4, 128], f32, tag="qk4",
                                    bufs=3 if "tpsmerge" in opts else None)
                                for qi in range(4):
                                    n0 = 32 * a + 8 * qi
                                    q_v = qkv_sb["q"][:, n0:n0 + 8, :].rearrange(
                                        "d g h -> d (g h)")
                                    k_v = qkv_sb["k"][:, n0:n0 + 8, :].rearrange(
                                        "d g h -> d (g h)")
                                    nc.tensor.matmul(
                                        qk4[:, qi, :], lhsT=q_v, rhs=k_v,
                                        start=(qi == 0), stop=False,
                                        skip_group_check=True)
                                    nc.tensor.matmul(
                                        qk4[:, qi, :], lhsT=ml_sb, rhs=mr_sb,
                                        start=False, stop=(qi == 3),
                                        skip_group_check=True)
                                att4 = attp.tile([128, 4, 128], bf16, tag="att4")
                                nc.scalar.activation(att4, qk4, Exp, bias=ebias)
                                rs4 = attp.tile([128, 4], f32, tag="rs4")
                                nc.vector.reduce_sum(
                                    out=rs4, in_=att4,
                                    axis=mybir.AxisListType.X)
                                rc4 = attp.tile([128, 4], f32, tag="rc4")
                                nc.vector.reciprocal(rc4, rs4)
                                for qi in range(4):
                                    if "mulact" in opts:
                                        nc.scalar.mul(
                                            att4[:, qi, :], att4[:, qi, :],
                                            rc4[:, qi:qi + 1])
                                    else:
                                        nc.vector.tensor_scalar_mul(
                                            att4[:, qi, :], att4[:, qi, :],
                                            rc4[:, qi:qi + 1])

                                if "tpsmerge" in opts:
                                    tps_ps = ps2.tile(
                                        [128, 8, 128], bf16, tag="tps",
                                        bufs=2)
                                    for qi in range(4):
                                        n0 = 32 * a + 8 * qi
                                        v_v = qkv_sb["v"][
                                            :, n0:n0 + 8, :].rearrange(
                                            "d g t -> d (g t)")
                                        nc.tensor.matmul(
                                            tps_ps[:, 4 + qi, :], lhsT=v_v,
                                            rhs=ident, is_transpose=True,
                                            start=(qi == 0), stop=False,
                                            skip_group_check=True)
                                    for qi in range(4):
                                        nc.tensor.matmul(
                                            tps_ps[:, qi, :],
                                            lhsT=att4[:, qi, :], rhs=ident,
                                            is_transpose=True,
                                            start=False, stop=(qi == 3),
                                            skip_group_check=True)
                                    tps = attp.tile(
                                        [128, 8, 128], bf16, tag="tpss")
                                    if "attcopyact" in opts:
                                        nc.scalar.copy(tps, tps_ps)
                                    else:
                                        nc.vector.tensor_copy(tps, tps_ps)
                                    attT4 = tps[:, 0:4, :]
                                    vp4 = tps[:, 4:8, :]
                                else:
                                    attT4_ps = ps2.tile(
                                        [128, 4, 128], bf16, tag="attT4")
                                    vp4_ps = ps2.tile(
                                        [128, 4, 128], bf16, tag="vp4")
                                    for qi in range(4):
                                        n0 = 32 * a + 8 * qi
                                        nc.tensor.matmul(
                                            attT4_ps[:, qi, :],
                                            lhsT=att4[:, qi, :], rhs=ident,
                                            is_transpose=True,
                                            start=(qi == 0), stop=(qi == 3),
                                            skip_group_check=True)
                                        v_v = qkv_sb["v"][
                                            :, n0:n0 + 8, :].rearrange(
                                            "d g t -> d (g t)")
                                        nc.tensor.matmul(
                                            vp4_ps[:, qi, :], lhsT=v_v,
                                            rhs=ident, is_transpose=True,
                                            start=(qi == 0), stop=(qi == 3),
                                            skip_group_check=True)
                                    attT4 = attp.tile(
                                        [128, 4, 128], bf16, tag="attT4s")
                                    if "attcopyact" in opts:
                                        nc.scalar.copy(attT4, attT4_ps)
                                    else:
                                        nc.vector.tensor_copy(attT4, attT4_ps)
                                    vp4 = attp.tile(
                                        [128, 4, 128], bf16, tag="vp4s")
                                    if "vpcopyact" in opts:
                                        nc.scalar.copy(vp4, vp4_ps)
                                    else:
                                        nc.vector.tensor_copy(vp4, vp4_ps)

                                o4_ps = ps2.tile(
                                    [128, 4, 128], f32, tag="o4",
                                    bufs=3 if "tpsmerge" in opts else None)
                                for qi in range(4):
                                    nc.tensor.matmul(
                                        o4_ps[:, qi, :],
                                        lhsT=vp4[:, qi, :],
                                        rhs=attT4[:, qi, :],
                                        start=(qi == 0), stop=(qi == 3),
                                        skip_group_check=True)
                                tc0 = 2 * a
                                half, tcl0 = tc0 // tw2, tc0 % tw2
                                dst = scr5[half][:, :, tcl0:tcl0 + 2, :].rearrange(
                                    "p (jb g) t h -> p t jb g h", jb=2)
                                nc.vector.tensor_copy(
                                    dst,
                                    o4_ps.rearrange(
                                        "p (tb jb) (g h) -> p tb jb g h",
                                        jb=2, h=16))
                            nquads_left = 0
                        else:
                            nquads_left = nquads
                        for g0 in range(
                                0 if "noatt" not in opts and nquads_left else 10**9,
                                nquads_left):
                            n0 = 8 * g0
                            j0, tc_ = n0 % 16, g0 // 2
                            q_v = qkv_sb["q"][:, n0:n0 + 8, :].rearrange(
                                "d g h -> d (g h)")
                            k_v = qkv_sb["k"][:, n0:n0 + 8, :].rearrange(
                                "d g h -> d (g h)")
                            qk_ps = ps2.tile([128, 128], f32, tag="qk")
                            att = attp.tile([128, 128], bf16, tag="att")
                            rs = attp.tile([128, 1], f32, tag="rs")
                            if "fastmask" in opts:
                                nc.tensor.matmul(
                                    qk_ps, lhsT=q_v, rhs=k_v,
                                    start=True, stop=False)
                                nc.tensor.matmul(
                                    qk_ps, lhsT=ml_sb, rhs=mr_sb,
                                    start=False, stop=True)
                                nc.scalar.activation(
                                    att, qk_ps, Exp, bias=ebias, accum_out=rs)
                            else:
                                nc.tensor.matmul(
                                    qk_ps, lhsT=q_v, rhs=k_v,
                                    start=True, stop=True)
                                if "noatt_dve" not in opts:
                                    nc.vector.tensor_add(qk_ps, qk_ps, mask_sb)
                                nc.scalar.activation(att, qk_ps, Exp, accum_out=rs)
                            if "noatt_dve" not in opts:
                                if "divnorm" in opts:
                                    nc.vector.tensor_scalar(
                                        att, att, rs, None,
                                        op0=mybir.AluOpType.divide)
                                else:
                                    rc = attp.tile([128, 1], f32, tag="rc")
                                    nc.vector.reciprocal(rc, rs)
                                    nc.vector.tensor_scalar_mul(att, att, rc)

                            attT_ps = ps2.tile([128, 128], bf16, tag="attT")
                            nc.tensor.transpose(attT_ps, att, ident)
                            attT = attp.tile([128, 128], bf16, tag="attTs")
                            nc.vector.tensor_copy(attT, attT_ps)

                            if vphoist:
                                vp = v2_sb[:, g0, :]
                            else:
                                v_v = qkv_sb["v"][:, n0:n0 + 8, :].rearrange(
                                    "d g t -> d (g t)")
                                vp_ps = ps2.tile([128, 128], bf16, tag="vp")
                                nc.tensor.transpose(vp_ps, v_v, ident)
                                vp = attp.tile([128, 128], bf16, tag="vps")
                                nc.vector.tensor_copy(vp, vp_ps)

                            o_ps = ps2.tile(
                                [128, 128], f32, tag="o",
                                bufs=2 if vphoist else None)
                            nc.tensor.matmul(
                                o_ps, lhsT=vp, rhs=attT, start=True, stop=True)
                            if "scrsplit" in opts:
                                tw2 = TW // 2
                                half, tcl = tc_ // tw2, tc_ % tw2
                                dst = scr5[half][:, j0:j0 + 8, tcl:tcl + 1, :]
                            else:
                                dst = scr4[:, j0:j0 + 8, :, tc_:tc_ + 1]
                            nc.vector.tensor_copy(
                                dst, o_ps.rearrange("p (g h) -> p g h", g=8))

            # ---------------- phase 3: output projection ----------------
            wo_v = w_d["o"].ap().rearrange("(jt jp) r -> jp jt r", jp=128)
            out_v = out_d.ap().rearrange("(rt rp) n -> rp rt n", rp=128)
            if "scrsplit" in opts:
                ch3 = npos // 2
                chunks = [(scrA, slice(0, ch3)), (scrB, slice(ch3, npos))]
            else:
                ch3 = CH
                chunks = [
                    (scr_sb, slice(c * CH, (c + 1) * CH)) for c in range(NCH)]
            with tc.tile_pool(name="ps3", bufs=3, space="PSUM") as ps3:
                for rg in range(8):
                    w_sb = wpool.tile([128, 16, 256], bf16, tag="w")
                    nc.sync.dma_start(w_sb, wo_v[:, :, rg * 256:(rg + 1) * 256])
                    for ro in range(2):
                        r = rg * 2 + ro
                        for src, csl in chunks:
                            ps = ps3.tile([128, ch3], f32)
                            for j in range(16):
                                rhs = (src[:, j, :] if "scrsplit" in opts
                                       else src[:, j, csl])
                                nc.tensor.matmul(
                                    ps,
                                    lhsT=w_sb[:, j, ro * 128:(ro + 1) * 128],
                                    rhs=rhs,
                                    start=(j == 0), stop=(j == 15))
                            o_sb = outp.tile([128, ch3], f32)
                            nc.scalar.activation(
                                o_sb, ps, Ident, bias=b_sb["o"][:, r:r + 1])
                            nc.sync.dma_start(out_v[:, r, csl], o_sb)

    with tile.TileContext(nc) as tc:
        for _ in range(reps):
            body(tc)

    nc.compile()
    _NC_CACHE[key] = nc
    return nc


# ---------------------------------------------------------------------------
# Runner (PJRT via axon, cached jitted callable)
# ---------------------------------------------------------------------------

_RUNNER_CACHE = {}


def make_runner(nc, ncores=NCORES):
    """Returns run(in_maps) -> list of per-core output dicts.

    Mirrors bass2jax.run_bass_via_pjrt but caches the jitted callable and
    does NOT donate output buffers (kernel writes every output element), so
    repeated timed calls don't re-trace or re-transfer.
    """
    key = id(nc)
    if key in _RUNNER_CACHE:
        return _RUNNER_CACHE[key]

    import jax
    import numpy as _np
    from jax.sharding import Mesh, PartitionSpec
    from jax.experimental.shard_map import shard_map
    from concourse import mybir
    from concourse import bass2jax
    from concourse.bass2jax import (
        _bass_exec_p, install_neuronx_cc_hook, partition_id_tensor)

    install_neuronx_cc_hook()

    partition_name = (
        nc.partition_id_tensor.name if nc.partition_id_tensor else None)
    in_names, out_names, out_avals, zero_outs = [], [], [], []
    for alloc in nc.m.functions[0].allocations:
        if not isinstance(alloc, mybir.MemoryLocationSet):
            continue
        name = alloc.memorylocations[0].name
        if alloc.kind == "ExternalInput":
            if name == partition_name:
                continue
            in_names.append(name)
        elif alloc.kind == "ExternalOutput":
            shape = tuple(alloc.tensor_shape)
            dtype = mybir.dt.np(alloc.dtype)
            out_names.append(name)
            out_avals.append(jax.core.ShapedArray(shape, dtype))
            zero_outs.append(_np.zeros(shape, dtype))
    n_params = len(in_names)
    all_in_names = in_names + out_names
    if partition_name is not None:
        all_in_names = all_in_names + [partition_name]

    def _body(*args):
        operands = list(args)
        if partition_name is not None:
            operands.append(partition_id_tensor())
        outs = _bass_exec_p.bind(
            *operands,
            out_avals=tuple(out_avals),
            in_names=tuple(all_in_names),
            out_names=tuple(out_names),
            lowering_input_output_aliases=(),
            sim_require_finite=True,
            sim_require_nnan=True,
            nc=nc,
        )
        return tuple(outs)

    devices = jax.devices()[:ncores]
    mesh = Mesh(np.asarray(devices), ("core",))
    n_outs = len(out_names)
    jitted = jax.jit(
        shard_map(
            _body, mesh=mesh,
            in_specs=(PartitionSpec("core"),) * (n_params + n_outs),
            out_specs=(PartitionSpec("core"),) * n_outs,
            check_rep=False,
        ),
        keep_unused=True,
    )

    zeros_dev = [
        jax.device_put(
            _np.zeros((ncores * z.shape[0], *z.shape[1:]), z.dtype))
        for z in zero_outs
    ]

    def put(in_maps):
        concat = [
            _np.concatenate([_np.asarray(m[name]) for m in in_maps], axis=0)
            for name in in_names
        ]
        return [jax.device_put(a) for a in concat]

    def run_dev(in_dev):
        outs = jitted(*in_dev, *zeros_dev)
        jax.block_until_ready(outs)
        return outs

    def run(in_maps):
        outs = run_dev(put(in_maps))
        res = []
        for c in range(len(in_maps)):
            res.append({
                name: _np.asarray(outs[i]).reshape(
                    len(in_maps), *out_avals[i].shape)[c]
                for i, name in enumerate(out_names)
            })
        return res

    run.put = put
    run.run_dev = run_dev
    run.out_names = out_names
    _RUNNER_CACHE[key] = run
    return run


DEFAULT_OPTS = frozenset(
    {"fastmask", "scrsplit", "qbatch", "attcopyact", "vpcopyact"})


def kernel(**inputs) -> np.ndarray:
    in_maps, meta = prepare_host(**{k: np.asarray(v) for k, v in inputs.items()})
    nc = build_nc(NPOS, opts=DEFAULT_OPTS)
    run = make_runner(nc, NCORES)
    outs = run(in_maps)
    layout = "t_h" if "scrsplit" in DEFAULT_OPTS else "h_t"
    return assemble_output(outs, meta, NPOS, layout)
